# revision 1
# baseline (speedup 1.0000x reference)
"""GraphTransformerLayer on 8 Trainium2 NeuronCores (Bass/Tile).

Sharding: 8-way along the query-node axis. Each core owns NQ=512 query rows,
computes the full K/V projections (replicated), its slice of the masked
attention, and its slice of the FFN. No collectives needed; the host
concatenates the 8 output slices.

Attention dataflow (per core, per head h):
  sT[m, n]  = sum_d k[m, d] q[n, d]        (PE; K=dk=64, out [128m, 512n])
  aT        = exp(0.125 * sT)              (ACT, PSUM->SBUF bf16)
  aT       *= maskT[m, n]                  (DVE, bf16)
  ctxT_ext  = [v_h | 1].T @ aT             (PE; out [65, 512n], row 64 = rowsum)
  ctxT_h    = ctxT_ext[0:64] * (1/rowsum)  (DVE; recip bcast via GPSIMD)
ctxT is directly the lhsT for the Wo matmul. h1/h2 residual+LN in f32.
"""

import sys

if "/opt/trn_rl_repo" not in sys.path:
    sys.path.insert(0, "/opt/trn_rl_repo")

import numpy as np
import ml_dtypes

import concourse.bacc as bacc
import concourse.tile as tile
import concourse.mybir as mybir
from concourse.bass_utils import run_bass_kernel_spmd

BF16 = ml_dtypes.bfloat16
F32 = mybir.dt.float32
BF = mybir.dt.bfloat16

N = 4096
D = 512
H = 8
DK = 64
DFF = 2048
NCORES = 8
NQ = N // NCORES  # 512 query rows per core
P = 128
EPS = 1e-5

ALU = mybir.AluOpType
AF = mybir.ActivationFunctionType

# set by test.py to capture a profile
TRACE = False
TRACE_DIR = None
LAST_EXEC_NS = None

# debug: truncate the program after a phase (1=proj, 2=attention, 3=ln1, 4=full)
STOP_AT = 4

_CACHED = None


def _build():
    nc = bacc.Bacc("TRN2", target_bir_lowering=False, debug=False,
                   num_devices=NCORES)

    # ---- DRAM I/O ----
    hT = nc.dram_tensor("hT", [D, N], BF, kind="ExternalInput").ap()
    hqT = nc.dram_tensor("hqT", [D, NQ], BF, kind="ExternalInput").ap()
    hq = nc.dram_tensor("hq", [NQ, D], F32, kind="ExternalInput").ap()
    maskT = nc.dram_tensor("maskT", [N, NQ], BF, kind="ExternalInput").ap()
    wqT = nc.dram_tensor("wqT", [D, D], BF, kind="ExternalInput").ap()
    wkT = nc.dram_tensor("wkT", [D, D], BF, kind="ExternalInput").ap()
    wvT = nc.dram_tensor("wvT", [D, D], BF, kind="ExternalInput").ap()
    woT = nc.dram_tensor("woT", [D, D], BF, kind="ExternalInput").ap()
    w1T = nc.dram_tensor("w1T", [D, DFF], BF, kind="ExternalInput").ap()
    w2T = nc.dram_tensor("w2T", [DFF, D], BF, kind="ExternalInput").ap()
    bq = nc.dram_tensor("bq", [D], F32, kind="ExternalInput").ap()
    bk = nc.dram_tensor("bk", [D], F32, kind="ExternalInput").ap()
    b1 = nc.dram_tensor("b1", [DFF], F32, kind="ExternalInput").ap()
    bv2 = nc.dram_tensor("bv2", [1, D], BF, kind="ExternalInput").ap()
    b22 = nc.dram_tensor("b22", [1, D], BF, kind="ExternalInput").ap()
    g1b = nc.dram_tensor("g1b", [P, D], F32, kind="ExternalInput").ap()
    b1b = nc.dram_tensor("b1b", [P, D], F32, kind="ExternalInput").ap()
    g2b = nc.dram_tensor("g2b", [P, D], F32, kind="ExternalInput").ap()
    b2b = nc.dram_tensor("b2b", [P, D], F32, kind="ExternalInput").ap()
    ident = nc.dram_tensor("ident", [P, P], F32, kind="ExternalInput").ap()
    out = nc.dram_tensor("out", [NQ, D], F32, kind="ExternalOutput").ap()

    with tile.TileContext(nc) as tc:
        _emit(nc, tc, locals())
    nc.compile()
    return nc


def _emit(nc, tc, t):
    hT, hqT, hq, maskT = t["hT"], t["hqT"], t["hq"], t["maskT"]
    wqT, wkT, wvT, woT = t["wqT"], t["wkT"], t["wvT"], t["woT"]
    w1T, w2T = t["w1T"], t["w2T"]
    bq, bk, b1, bv2, b22 = t["bq"], t["bk"], t["b1"], t["bv2"], t["b22"]
    g1b, b1b, g2b, b2b = t["g1b"], t["b1b"], t["g2b"], t["b2b"]
    ident, out = t["ident"], t["out"]

    from contextlib import ExitStack

    es = ExitStack()
    with es:
        cpool = es.enter_context(tc.tile_pool(name="const", bufs=1))
        h1pool = es.enter_context(tc.tile_pool(name="h1p", bufs=1))
        qkv_es = ExitStack()
        mpool = qkv_es.enter_context(tc.tile_pool(name="maskp", bufs=1))
        qkvpool = qkv_es.enter_context(tc.tile_pool(name="qkvp", bufs=1))

        # ---- constants ----
        bq_sb = cpool.tile([P, 4], F32, tag="bq")
        nc.sync.dma_start(bq_sb[:], bq.rearrange("(t p) -> p t", p=P))
        bk_sb = cpool.tile([P, 4], F32, tag="bk")
        nc.sync.dma_start(bk_sb[:], bk.rearrange("(t p) -> p t", p=P))
        b1_sb = cpool.tile([P, 16], F32, tag="b1")
        nc.sync.dma_start(b1_sb[:], b1.rearrange("(t p) -> p t", p=P))
        bv_sb = cpool.tile([1, D], BF, tag="bv")
        nc.sync.dma_start(bv_sb[:], bv2[:])
        b2_sb = cpool.tile([1, D], BF, tag="b2")
        nc.sync.dma_start(b2_sb[:], b22[:])
        ident_sb = cpool.tile([P, P], F32, tag="id")
        nc.sync.dma_start(ident_sb[:], ident[:])
        identb_sb = cpool.tile([P, P], BF, tag="idb")
        nc.vector.tensor_copy(identb_sb[:], ident_sb[:])
        ones_sb = cpool.tile([1, P], BF, tag="ones")
        nc.vector.memset(ones_sb[:], 1.0)
        eps_sb = cpool.tile([P, 1], F32, tag="eps")
        nc.vector.memset(eps_sb[:], EPS)
        ln_sb = {}
        for nm, src in (("g1", g1b), ("b1l", b1b), ("g2", g2b), ("b2l", b2b)):
            tl = cpool.tile([P, D], F32, tag=nm, name=f"ln_{nm}")
            nc.sync.dma_start(tl[:], src[:])
            ln_sb[nm] = tl
        hq_sb = []
        for qt in range(4):
            tl = cpool.tile([P, D], F32, tag=f"hq{qt}", name=f"hq{qt}")
            nc.sync.dma_start(tl[:], hq[qt * P:(qt + 1) * P, :])
            hq_sb.append(tl)
        woT_sb = []
        for s in range(4):
            tl = cpool.tile([P, D], BF, tag=f"wo{s}", name=f"wo{s}")
            nc.sync.dma_start(tl[:], woT[s * P:(s + 1) * P, :])
            woT_sb.append(tl)

        # ---- persistent qkv outputs ----
        kT_sb = [qkvpool.tile([P, N], BF, tag=f"kt{i}", name=f"kT{i}")
                 for i in range(4)]
        qT_sb = [qkvpool.tile([P, NQ], BF, tag=f"qt{i}", name=f"qT{i}")
                 for i in range(4)]
        # v_ext[mt]: [128, 8*65]; per head h cols h*65..h*65+64, col 64 = ones
        v_sb = [qkvpool.tile([P, H * (DK + 1)], BF, tag=f"v{i}", name=f"v{i}")
                for i in range(32)]

        # ================= projections =================
        with tc.tile_pool(name="projp", bufs=1) as ppool, \
             tc.tile_pool(name="psproj", bufs=4, space="PSUM") as psp:
            wq_sb, wk_sb, wv_sb, hqT_sb = [], [], [], []
            for s in range(4):
                tl = ppool.tile([P, D], BF, tag=f"wq{s}", name=f"wq{s}")
                nc.sync.dma_start(tl[:], wqT[s * P:(s + 1) * P, :])
                wq_sb.append(tl)
                tl = ppool.tile([P, D], BF, tag=f"wk{s}", name=f"wk{s}")
                nc.sync.dma_start(tl[:], wkT[s * P:(s + 1) * P, :])
                wk_sb.append(tl)
                tl = ppool.tile([P, D], BF, tag=f"wv{s}", name=f"wv{s}")
                nc.sync.dma_start(tl[:], wvT[s * P:(s + 1) * P, :])
                wv_sb.append(tl)
                tl = ppool.tile([P, NQ], BF, tag=f"hqT{s}", name=f"hqT{s}")
                nc.sync.dma_start(tl[:], hqT[s * P:(s + 1) * P, :])
                hqT_sb.append(tl)

            # mask DMA issued after the projection inputs so the first
            # matmuls are not stuck behind a 4MB transfer
            mask_sb = mpool.tile([P, 32 * NQ], BF, tag="mask")
            nc.gpsimd.dma_start(
                mask_sb.rearrange("p (mt j) -> p mt j", j=NQ),
                maskT.rearrange("(mt p) j -> p mt j", p=P),
            )

            # qT[t] = (Wq @ hqT)[t-rows] + bq
            for tt in range(4):
                ps = psp.tile([P, NQ], F32, tag="pp", name="ps_q")
                for s in range(4):
                    nc.tensor.matmul(ps[:], wq_sb[s][:, tt * P:(tt + 1) * P],
                                     hqT_sb[s][:], start=(s == 0), stop=(s == 3))
                nc.scalar.activation(qT_sb[tt][:], ps[:], AF.Identity,
                                     bias=bq_sb[:, tt:tt + 1])

            # kT and v, streaming hT in two halves of 2048 columns
            for half in range(2):
                c0 = half * (N // 2)
                hT_sb = []
                for s in range(4):
                    tl = ppool.tile([P, N // 2], BF, tag=f"ht{s}", name=f"ht{s}")
                    nc.sync.dma_start(tl[:], hT[s * P:(s + 1) * P,
                                                c0:c0 + N // 2])
                    hT_sb.append(tl)
                for tt in range(4):
                    for c in range(4):
                        ps = psp.tile([P, 512], F32, tag="pp", name="ps_k")
                        for s in range(4):
                            nc.tensor.matmul(
                                ps[:], wk_sb[s][:, tt * P:(tt + 1) * P],
                                hT_sb[s][:, c * 512:(c + 1) * 512],
                                start=(s == 0), stop=(s == 3))
                        nc.vector.tensor_scalar_add(
                            kT_sb[tt][:, c0 + c * 512:c0 + (c + 1) * 512],
                            ps[:], bk_sb[:, tt:tt + 1])
                for mtl in range(16):
                    mt = half * 16 + mtl
                    ps = psp.tile([P, D], F32, tag="pp", name="ps_v")
                    for s in range(4):
                        nc.tensor.matmul(ps[:],
                                         hT_sb[s][:, mtl * P:(mtl + 1) * P],
                                         wv_sb[s][:], start=(s == 0),
                                         stop=False)
                    nc.tensor.matmul(ps[:], ones_sb[:], bv_sb[:],
                                     start=False, stop=True)
                    vv = v_sb[mt].rearrange("p (h c) -> p h c", c=DK + 1)
                    nc.scalar.copy(vv[:, :, 0:DK],
                                   ps.rearrange("p (h c) -> p h c", c=DK))
                    nc.vector.memset(vv[:, :, DK:DK + 1], 1.0)

        if STOP_AT == 1:
            # anchor projections: out[qt] = f32(kT[qt][:, :512] + qT) + v
            for qt in range(4):
                cv = h1pool.tile([P, D], F32, tag=f"x{qt}", bufs=2, name="cv")
                nc.vector.tensor_add(cv[:], kT_sb[qt][:, 0:D], qT_sb[qt][:])
                nc.vector.tensor_add(cv[:], cv[:], v_sb[qt * 8][:, 0:D])
                nc.sync.dma_start(out[qt * P:(qt + 1) * P, :], cv[:])
            qkv_es.close()
            return

        # ================= attention =================
        with tc.tile_pool(name="attp", bufs=1) as apool:
            ctxT_sb = [apool.tile([P, NQ], BF, tag=f"cx{i}", name=f"ctxT{i}")
                       for i in range(4)]
            with tc.tile_pool(name="psatt", bufs=1, space="PSUM") as psa:
                for hp in range(4):
                    h0, h1_ = 2 * hp, 2 * hp + 1
                    ctx_ps = [psa.tile([P, NQ], F32, tag="pc", bufs=2,
                                       name="ctx_ps") for _ in range(2)]
                    for g in range(16):
                        sp = [psa.tile([P, 1024], F32, tag="ps", bufs=3,
                                       name="sc_ps") for _ in range(2)]
                        at = [apool.tile([P, 1024], BF, tag="at", bufs=6,
                                         name="at") for _ in range(2)]
                        for i, po in ((0, 0), (1, DK)):
                            for j in range(2):
                                mt = 2 * g + j
                                nc.tensor.matmul(
                                    sp[i][:, j * NQ:(j + 1) * NQ],
                                    kT_sb[hp][po:po + DK, mt * P:(mt + 1) * P],
                                    qT_sb[hp][po:po + DK, :],
                                    start=True, stop=True)
                        for i in range(2):
                            nc.scalar.activation(at[i][:], sp[i][:], AF.Exp,
                                                 scale=0.125)
                            nc.vector.tensor_mul(
                                at[i][:], at[i][:],
                                mask_sb[:, g * 1024:(g + 1) * 1024])
                        for i, h in ((0, h0), (1, h1_)):
                            for j in range(2):
                                mt = 2 * g + j
                                nc.tensor.matmul(
                                    ctx_ps[i][0:DK + 1, :],
                                    v_sb[mt][:, h * 65:h * 65 + 65],
                                    at[i][:, j * NQ:(j + 1) * NQ],
                                    start=(mt == 0), stop=(mt == 31))
                    # normalize: ctxT_h = ctx[0:64] * (1/rowsum)
                    for i, po in ((0, 0), (1, DK)):
                        rec = apool.tile([1, NQ], F32, tag="rec", bufs=2,
                                         name="rec")
                        nc.vector.reciprocal(rec[:], ctx_ps[i][DK:DK + 1, :])
                        bc = apool.tile([P, NQ], F32, tag="bc", bufs=2,
                                        name="bc")
                        nc.gpsimd.partition_broadcast(bc[:], rec[:])
                        nc.vector.tensor_mul(ctxT_sb[hp][po:po + DK, :],
                                             ctx_ps[i][0:DK, :],
                                             bc[0:DK, :])

            if STOP_AT == 2:
                for qt in range(4):
                    cv = h1pool.tile([P, D], F32, tag=f"x{qt}", bufs=2,
                                     name="cv")
                    nc.vector.tensor_copy(cv[:], ctxT_sb[qt][:])
                    nc.sync.dma_start(out[qt * P:(qt + 1) * P, :], cv[:])

            # ---- Wo + residual + LN1 + transpose ----
            h1_sb = [h1pool.tile([P, D], F32, tag=f"h1_{i}", name=f"h1_{i}")
                     for i in range(4)]
            h1T_sb = [h1pool.tile([P, NQ], BF, tag=f"h1T{i}", name=f"h1T{i}")
                      for i in range(4)]
            with tc.tile_pool(name="pspost", bufs=2, space="PSUM") as psw:
                for qt in range(4 if STOP_AT > 2 else 0):
                    ps = psw.tile([P, D], F32, tag="po", name="wo_ps")
                    for s in range(4):
                        nc.tensor.matmul(ps[:],
                                         ctxT_sb[s][:, qt * P:(qt + 1) * P],
                                         woT_sb[s][:], start=(s == 0),
                                         stop=(s == 3))
                    if STOP_AT == 30:
                        nc.vector.tensor_add(h1_sb[qt][:], ps[:], hq_sb[qt][:])
                        continue
                    h1 = _layer_norm(nc, h1pool, qt, ps, hq_sb[qt],
                                     ln_sb["g1"], ln_sb["b1l"], h1_sb[qt],
                                     eps_sb, stop_at=STOP_AT)
                    if STOP_AT in (31, 32, 311, 312, 313, 3110, 3111):
                        continue
                    h1b = h1pool.tile([P, D], BF, tag="h1b", bufs=2,
                                      name="h1b")
                    nc.vector.tensor_copy(h1b[:], h1[:])
                    for i in range(4):
                        tp = psw.tile([P, P], BF, tag="tp", name="tp")
                        nc.tensor.transpose(tp[:], h1b[:, i * P:(i + 1) * P],
                                            identb_sb[:])
                        nc.vector.tensor_copy(
                            h1T_sb[i][:, qt * P:(qt + 1) * P], tp[:])

        qkv_es.close()

        if STOP_AT == 2:
            return
        if STOP_AT in (3, 30, 31, 32, 311, 312, 313, 3110, 3111):
            for qt in range(4):
                nc.sync.dma_start(out[qt * P:(qt + 1) * P, :], h1_sb[qt][:])
            return

        # ================= FFN =================
        with tc.tile_pool(name="ffnp", bufs=1) as fpool, \
             tc.tile_pool(name="psffn", bufs=4, space="PSUM") as psf:
            w1_sb = []
            for s in range(4):
                tl = fpool.tile([P, DFF], BF, tag=f"w1_{s}", name=f"w1_{s}")
                nc.sync.dma_start(tl[:], w1T[s * P:(s + 1) * P, :])
                w1_sb.append(tl)
            w2_sb = []
            for ft in range(16):
                tl = fpool.tile([P, D], BF, tag=f"w2_{ft}", name=f"w2_{ft}")
                nc.sync.dma_start(tl[:], w2T[ft * P:(ft + 1) * P, :])
                w2_sb.append(tl)
            fT_sb = [fpool.tile([P, NQ], BF, tag=f"fT{i}", name=f"fT{i}")
                     for i in range(16)]
            for ft in range(16):
                ps = psf.tile([P, NQ], F32, tag="pf", name="f_ps")
                for s in range(4):
                    nc.tensor.matmul(ps[:], w1_sb[s][:, ft * P:(ft + 1) * P],
                                     h1T_sb[s][:], start=(s == 0),
                                     stop=(s == 3))
                nc.scalar.activation(fT_sb[ft][:], ps[:], AF.Relu,
                                     bias=b1_sb[:, ft:ft + 1])
            for qt in range(4):
                ps = psf.tile([P, D], F32, tag="pf", name="ff_ps")
                for ft in range(16):
                    nc.tensor.matmul(ps[:], fT_sb[ft][:, qt * P:(qt + 1) * P],
                                     w2_sb[ft][:], start=(ft == 0), stop=False)
                nc.tensor.matmul(ps[:], ones_sb[:], b2_sb[:],
                                 start=False, stop=True)
                h2 = _layer_norm(nc, h1pool, qt + 4, ps, h1_sb[qt],
                                 ln_sb["g2"], ln_sb["b2l"], None, eps_sb)
                nc.sync.dma_start(out[qt * P:(qt + 1) * P, :], h2[:])


def _layer_norm(nc, pool, uid, z_ps, res_sb, g_sb, b_sb, out_tile, eps_sb,
                stop_at=4):
    """out = LN(z_ps + res_sb) * g + b, f32. Returns the output tile."""
    x = pool.tile([P, D], F32, tag=f"x{uid % 4}", bufs=2, name=f"x{uid}")
    s1 = pool.tile([P, 1], F32, tag="s1", bufs=4, name="s1")
    nc.vector.scalar_tensor_tensor(x[:], z_ps[:], 0.0, res_sb[:],
                                   op0=ALU.add, op1=ALU.add, accum_out=s1[:])
    if stop_at == 3110:
        nc.vector.tensor_scalar_add(out_tile[:], x[:], s1[:])
        return out_tile
    xsq = pool.tile([P, D], F32, tag="xsq", bufs=2, name="xsq")
    s2 = pool.tile([P, 1], F32, tag="s2", bufs=4, name="s2")
    nc.vector.tensor_mul(xsq[:], x[:], x[:])
    nc.vector.reduce_sum(s2[:], xsq[:], axis=mybir.AxisListType.X)
    if stop_at == 3111:
        nc.vector.tensor_scalar_add(out_tile[:], xsq[:], s2[:])
        return out_tile
    if stop_at == 311:
        nc.vector.tensor_scalar(out_tile[:], xsq[:], s2[:], s1[:],
                                op0=ALU.add, op1=ALU.add)
        return out_tile
    nm = pool.tile([P, 1], F32, tag="nm", bufs=4, name="nm")
    nc.vector.tensor_scalar_mul(nm[:], s1[:], -1.0 / D)
    m2 = pool.tile([P, 1], F32, tag="m2", bufs=4, name="m2")
    nc.vector.tensor_mul(m2[:], nm[:], nm[:])
    var = pool.tile([P, 1], F32, tag="var", bufs=4, name="var")
    nc.vector.scalar_tensor_tensor(var[:], s2[:], 1.0 / D, m2[:],
                                   op0=ALU.mult, op1=ALU.subtract)
    if stop_at == 312:
        nc.vector.tensor_scalar(out_tile[:], x[:], var[:], None, op0=ALU.add)
        return out_tile
    # rstd = rsqrt(var + eps), pure DVE: bit-trick seed + 3 Newton steps
    I32 = mybir.dt.int32
    ve = pool.tile([P, 1], F32, tag="ve", bufs=4, name="ve")
    nc.vector.tensor_scalar_add(ve[:], var[:], eps_sb[:])
    rstd = pool.tile([P, 1], F32, tag="rstd", bufs=4, name="rstd")
    nc.vector.tensor_single_scalar(rstd[:].bitcast(I32), ve[:].bitcast(I32),
                                   1, op=ALU.arith_shift_right)
    nc.vector.tensor_single_scalar(rstd[:].bitcast(I32), rstd[:].bitcast(I32),
                                   0x5F3759DF, op=ALU.subtract)
    nc.vector.tensor_single_scalar(rstd[:].bitcast(I32), rstd[:].bitcast(I32),
                                   -1, op=ALU.mult)
    tq = pool.tile([P, 1], F32, tag="tq", bufs=4, name="tq")
    for _ in range(3):
        nc.vector.tensor_mul(tq[:], rstd[:], rstd[:])
        nc.vector.tensor_mul(tq[:], tq[:], ve[:])
        nc.vector.tensor_scalar_mul(tq[:], tq[:], -0.5)
        nc.vector.tensor_scalar_add(tq[:], tq[:], 1.5)
        nc.vector.tensor_mul(rstd[:], rstd[:], tq[:])
    if stop_at == 313:
        nc.vector.tensor_scalar(out_tile[:], x[:], rstd[:], None, op0=ALU.add)
        return out_tile
    # xn = (x - mean) * rstd, in place
    nc.vector.tensor_scalar_add(x[:], x[:], nm[:])
    nc.vector.tensor_scalar_mul(x[:], x[:], rstd[:])
    if stop_at == 31:
        nc.vector.tensor_copy(out_tile[:], x[:])
        return out_tile
    if out_tile is None:
        out_tile = pool.tile([P, D], F32, tag=f"x{uid % 4}", bufs=2,
                             name=f"h2_{uid}")
    nc.vector.tensor_mul(out_tile[:], x[:], g_sb[:])
    nc.vector.tensor_add(out_tile[:], out_tile[:], b_sb[:])
    return out_tile


def _prep_inputs(inputs):
    h = np.asarray(inputs["h"], np.float32)
    adj = np.asarray(inputs["adj"])
    f32 = np.float32

    def bf(x):
        return np.ascontiguousarray(np.asarray(x, np.float32).astype(BF16))

    hT_full = bf(h.T)
    adjb = (adj != 0)
    np.fill_diagonal(adjb, True)
    adjb_bf = adjb.astype(BF16)

    wq, wk, wv, wo = (np.asarray(inputs[k], f32)
                      for k in ("Wq", "Wk", "Wv", "Wo"))
    w1, w2 = np.asarray(inputs["W1"], f32), np.asarray(inputs["W2"], f32)
    shared = {
        "hT": hT_full,
        "wqT": bf(wq.T), "wkT": bf(wk.T), "wvT": bf(wv.T), "woT": bf(wo.T),
        "w1T": bf(w1.T), "w2T": bf(w2.T),
        "bq": np.ascontiguousarray(np.asarray(inputs["bq"], f32)),
        "bk": np.ascontiguousarray(np.asarray(inputs["bk"], f32)),
        "b1": np.ascontiguousarray(np.asarray(inputs["b1"], f32)),
        "bv2": bf(np.asarray(inputs["bv"], f32)[None, :]),
        "b22": bf(np.asarray(inputs["b2"], f32)[None, :]),
        "g1b": np.ascontiguousarray(
            np.broadcast_to(np.asarray(inputs["ln1_g"], f32), (P, D))),
        "b1b": np.ascontiguousarray(
            np.broadcast_to(np.asarray(inputs["ln1_b"], f32), (P, D))),
        "g2b": np.ascontiguousarray(
            np.broadcast_to(np.asarray(inputs["ln2_g"], f32), (P, D))),
        "b2b": np.ascontiguousarray(
            np.broadcast_to(np.asarray(inputs["ln2_b"], f32), (P, D))),
        "ident": np.eye(P, dtype=f32),
    }
    bo = np.asarray(inputs["bo"], f32)
    in_maps = []
    for i in range(NCORES):
        r0 = i * NQ
        m = dict(shared)
        m["hqT"] = np.ascontiguousarray(hT_full[:, r0:r0 + NQ])
        m["hq"] = np.ascontiguousarray(h[r0:r0 + NQ, :] + bo)
        m["maskT"] = np.ascontiguousarray(adjb_bf[r0:r0 + NQ, :].T)
        in_maps.append(m)
    return in_maps


def kernel(**inputs) -> np.ndarray:
    global _CACHED, LAST_EXEC_NS
    if _CACHED is None:
        _CACHED = _build()
    nc = _CACHED
    in_maps = _prep_inputs(inputs)
    kw = {}
    if TRACE:
        kw = dict(trace=True, tmpdir=TRACE_DIR)
    res = run_bass_kernel_spmd(nc, in_maps, list(range(NCORES)), **kw)
    LAST_EXEC_NS = res.exec_time_ns
    return np.concatenate([res.results[i]["out"] for i in range(NCORES)],
                          axis=0)



# revision 5
# speedup vs baseline: 1.0674x; 1.0674x over previous
"""GraphTransformerLayer on 8 Trainium2 NeuronCores (Bass/Tile).

Sharding: 8-way along the query-node axis. Each core owns NQ=512 query rows,
computes the full K/V projections (replicated), its slice of the masked
attention, and its slice of the FFN. No collectives; the host concatenates
the 8 output slices.

fp8 DoubleRow is used for every matmul whose per-instruction contraction can
reach 256 (2 k-slices per partition): Q/K/V projections (c=512), ctx=attn@V
(c=4096 over m), Wo (c=512), FFN1 (c=512), FFN2 (c=2048). Scores (c=64 per
head) stay bf16 — DoubleRow cannot help a 64-deep contraction.

Attention dataflow per core, head h, m-step s (256 nodes = 2 m-tiles):
  sp[m, (j n)]  = k_h[m]·q_h[n]           PE bf16, 2 matmuls [64c,128,512]
  at            = exp(0.125 * sp)         ACT, PSUM->SBUF bf16 [128,1024]
  at8           = at * maskT              DVE bf16*bf16 -> fp8 [128,(2,512)]
  ctx_ps[66]   += [v8|1] DR@ at8          PE fp8 DoubleRow [128x2c, 65, 512]
ctx row 64 is the softmax denominator; after 16 steps:
  rec = approx(1/ctx_ps[64]); bc = bcast; ctxT8 = ctx_ps[0:64]*bc (fp8)
Odd heads DMA-shift their ctxT8 to partitions 64:127 (engines cannot cross
partitions; DMA can). Wo/FFN consume the fp8 tiles with DoubleRow.
h1/h2 residual+LN in f32 (rsqrt via bit-trick Newton, no ACT table swap).
"""

import sys

if "/opt/trn_rl_repo" not in sys.path:
    sys.path.insert(0, "/opt/trn_rl_repo")

import numpy as np
import ml_dtypes

import concourse.bacc as bacc
import concourse.tile as tile
import concourse.mybir as mybir
from concourse.bass_utils import run_bass_kernel_spmd

BF16 = ml_dtypes.bfloat16
F8NP = ml_dtypes.float8_e4m3
F32 = mybir.dt.float32
BF = mybir.dt.bfloat16
F8 = mybir.dt.float8e4

N = 4096
D = 512
H = 8
DK = 64
DFF = 2048
NCORES = 8
NQ = N // NCORES  # 512 query rows per core
P = 128
EPS = 1e-5
NSTEP = 16  # m-steps of 256 nodes

ALU = mybir.AluOpType
AF = mybir.ActivationFunctionType
DR = mybir.MatmulPerfMode.DoubleRow

# set by test.py to capture a profile
TRACE = False
TRACE_DIR = None
LAST_EXEC_NS = None

_CACHED = None


def _build():
    nc = bacc.Bacc("TRN2", target_bir_lowering=False, debug=False,
                   num_devices=NCORES)

    # ---- DRAM I/O ----
    # fp8 tensors are host-prefolded into DoubleRow layout:
    #   x8[p, (g, j, cols)] = X[g*256 + j*128 + p, cols]
    h8 = nc.dram_tensor("h8", [P, 4 * N], F8, kind="ExternalInput").ap()
    hq8 = nc.dram_tensor("hq8", [P, 4 * NQ], F8, kind="ExternalInput").ap()
    hq = nc.dram_tensor("hq", [NQ, D], F32, kind="ExternalInput").ap()
    maskT = nc.dram_tensor("maskT", [N, NQ], BF, kind="ExternalInput").ap()
    wq8 = nc.dram_tensor("wq8", [P, 4 * D], F8, kind="ExternalInput").ap()
    wk8 = nc.dram_tensor("wk8", [P, 4 * D], F8, kind="ExternalInput").ap()
    wv8 = nc.dram_tensor("wv8", [P, 4 * D], F8, kind="ExternalInput").ap()
    # wo8[p, (s, j, e)] = Wo.T[c, e],  c = (4s + 2j + p//64)*64 + p%64
    wo8 = nc.dram_tensor("wo8", [P, 4 * D], F8, kind="ExternalInput").ap()
    w18 = nc.dram_tensor("w18", [P, 4 * DFF], F8, kind="ExternalInput").ap()
    # w28[p, (fp, j, e)] = W2.T[fp*256 + j*128 + p, e]
    w28 = nc.dram_tensor("w28", [P, 16 * D], F8, kind="ExternalInput").ap()
    bq = nc.dram_tensor("bq", [D], F32, kind="ExternalInput").ap()
    bk = nc.dram_tensor("bk", [D], F32, kind="ExternalInput").ap()
    b1 = nc.dram_tensor("b1", [DFF], F32, kind="ExternalInput").ap()
    b22 = nc.dram_tensor("b22", [1, D], BF, kind="ExternalInput").ap()
    g1b = nc.dram_tensor("g1b", [P, D], F32, kind="ExternalInput").ap()
    b1b = nc.dram_tensor("b1b", [P, D], F32, kind="ExternalInput").ap()
    g2b = nc.dram_tensor("g2b", [P, D], F32, kind="ExternalInput").ap()
    b2b = nc.dram_tensor("b2b", [P, D], F32, kind="ExternalInput").ap()
    identb = nc.dram_tensor("identb", [P, P], BF, kind="ExternalInput").ap()
    ones8v = nc.dram_tensor("ones8v", [P, 32], F8, kind="ExternalInput").ap()
    out = nc.dram_tensor("out", [NQ, D], F32, kind="ExternalOutput").ap()

    with tile.TileContext(nc) as tc:
        _emit(nc, tc, locals())
    nc.compile()
    return nc


def _emit(nc, tc, t):
    h8, hq8, hq, maskT = t["h8"], t["hq8"], t["hq"], t["maskT"]
    wq8, wk8, wv8, wo8 = t["wq8"], t["wk8"], t["wv8"], t["wo8"]
    w18, w28 = t["w18"], t["w28"]
    bq, bk, b1, b22 = t["bq"], t["bk"], t["b1"], t["b22"]
    g1b, b1b, g2b, b2b = t["g1b"], t["b1b"], t["g2b"], t["b2b"]
    identb, out = t["identb"], t["out"]
    ones8v = t["ones8v"]

    from contextlib import ExitStack

    es = ExitStack()
    with es:
        cpool = es.enter_context(tc.tile_pool(name="const", bufs=1))
        h1pool = es.enter_context(tc.tile_pool(name="h1p", bufs=1))
        qkv_es = ExitStack()
        mpool = qkv_es.enter_context(tc.tile_pool(name="maskp", bufs=1))
        qkvpool = qkv_es.enter_context(tc.tile_pool(name="qkvp", bufs=1))

        # ---- constants ----
        bq_sb = cpool.tile([P, 4], F32, tag="bq")
        nc.sync.dma_start(bq_sb[:], bq.rearrange("(t p) -> p t", p=P))
        bk_sb = cpool.tile([P, 4], F32, tag="bk")
        nc.sync.dma_start(bk_sb[:], bk.rearrange("(t p) -> p t", p=P))
        b1_sb = cpool.tile([P, 16], F32, tag="b1")
        nc.sync.dma_start(b1_sb[:], b1.rearrange("(t p) -> p t", p=P))
        b2_sb = cpool.tile([1, D], BF, tag="b2")
        nc.sync.dma_start(b2_sb[:], b22[:])
        identb_sb = cpool.tile([P, P], BF, tag="idb")
        nc.sync.dma_start(identb_sb[:], identb[:])
        ones_sb = cpool.tile([1, P], BF, tag="ones")
        nc.vector.memset(ones_sb[:], 1.0)
        ones8_sb = cpool.tile([P, 32], F8, tag="o8")
        nc.sync.dma_start(ones8_sb[:], ones8v[:])
        eps_sb = cpool.tile([P, 1], F32, tag="eps")
        nc.vector.memset(eps_sb[:], EPS)
        ln_sb = {}
        for nm, src in (("g1", g1b), ("b1l", b1b), ("g2", g2b), ("b2l", b2b)):
            tl = cpool.tile([P, D], F32, tag=nm, name=f"ln_{nm}")
            nc.sync.dma_start(tl[:], src[:])
            ln_sb[nm] = tl
        hq_sb = []
        for qt in range(4):
            tl = cpool.tile([P, D], F32, tag=f"hq{qt}", name=f"hq{qt}")
            nc.sync.dma_start(tl[:], hq[qt * P:(qt + 1) * P, :])
            hq_sb.append(tl)
        # wo8: 4 tiles [64, 2, NQ-cols...] -> stored [P, 4*D] = [p,(s,j,e)]
        wo8_sb = cpool.tile([P, 4 * D], F8, tag="wo8")
        nc.sync.dma_start(wo8_sb[:], wo8[:])

        # ---- persistent qkv outputs ----
        kT_sb = [qkvpool.tile([P, N], BF, tag=f"kt{i}", name=f"kT{i}")
                 for i in range(4)]
        qT_sb = [qkvpool.tile([P, NQ], BF, tag=f"qt{i}", name=f"qT{i}")
                 for i in range(4)]
        # v8[s]: [128, (j, h, x)] fp8; x: 0..63 v dims, 64 ones, 65 pad
        # (slot width 66 keeps dual-fp8 ldweights offsets/sizes even)
        VS = DK + 2
        v8_sb = [qkvpool.tile([P, 2 * H * VS], F8, tag=f"v{i}",
                              name=f"v8_{i}") for i in range(NSTEP)]

        # ================= projections =================
        with tc.tile_pool(name="projp", bufs=1) as ppool, \
             tc.tile_pool(name="psproj", bufs=6, space="PSUM") as psp:
            h8_sb = ppool.tile([P, 4 * N], F8, tag="h8")
            nc.sync.dma_start(h8_sb[:], h8[:])
            hq8_sb = ppool.tile([P, 4 * NQ], F8, tag="hq8")
            nc.sync.dma_start(hq8_sb[:], hq8[:])
            wq_sb = ppool.tile([P, 4 * D], F8, tag="wq8")
            nc.sync.dma_start(wq_sb[:], wq8[:])
            wk_sb = ppool.tile([P, 4 * D], F8, tag="wk8")
            nc.sync.dma_start(wk_sb[:], wk8[:])
            wv_sb = ppool.tile([P, 4 * D], F8, tag="wv8")
            nc.sync.dma_start(wv_sb[:], wv8[:])

            # mask DMA issued after the projection inputs so the first
            # matmuls are not stuck behind a 4MB transfer
            mask_sb = mpool.tile([P, 32 * NQ], BF, tag="mask")
            nc.gpsimd.dma_start(
                mask_sb.rearrange("p (mt j) -> p mt j", j=NQ),
                maskT.rearrange("(mt p) j -> p mt j", p=P),
            )

            h8v = h8_sb.rearrange("p (g j n) -> p g j n", g=2, j=2)
            hq8v = hq8_sb.rearrange("p (g j n) -> p g j n", g=2, j=2)
            wqv = wq_sb.rearrange("p (g j o) -> p g j o", g=2, j=2)
            wkv = wk_sb.rearrange("p (g j o) -> p g j o", g=2, j=2)
            wvv = wv_sb.rearrange("p (g j o) -> p g j o", g=2, j=2)

            # qT[t] = (Wq @ hqT)[t-rows] + bq
            for tt in range(4):
                ps = psp.tile([P, NQ], F32, tag="pp", name="ps_q")
                for g in range(2):
                    nc.tensor.matmul(ps[:], wqv[:, g, :, tt * P:(tt + 1) * P],
                                     hq8v[:, g], start=(g == 0), stop=(g == 1),
                                     perf_mode=DR)
                nc.scalar.activation(qT_sb[tt][:], ps[:], AF.Identity,
                                     bias=bq_sb[:, tt:tt + 1])

            # kT: channels tt*128.., all 4096 nodes in 512-col chunks
            for tt in range(4):
                for ch in range(8):
                    c0 = ch * 512
                    ps = psp.tile([P, 512], F32, tag="pp", name="ps_k")
                    for g in range(2):
                        nc.tensor.matmul(
                            ps[:], wkv[:, g, :, tt * P:(tt + 1) * P],
                            h8v[:, g, :, c0:c0 + 512],
                            start=(g == 0), stop=(g == 1), perf_mode=DR)
                    nc.scalar.activation(kT_sb[tt][:, c0:c0 + 512], ps[:],
                                         AF.Identity,
                                         bias=bk_sb[:, tt:tt + 1])

            # v: per m-step s and half j, out [128 m, 512 d] -> fp8 v8
            for s in range(NSTEP):
                v8v = v8_sb[s].rearrange("p (j h x) -> p j h x", j=2, x=VS)
                for j in range(2):
                    m0 = s * 256 + j * P
                    ps = psp.tile([P, D], F32, tag="pp", name="ps_v")
                    for g in range(2):
                        nc.tensor.matmul(ps[:], h8v[:, g, :, m0:m0 + P],
                                         wvv[:, g], start=(g == 0),
                                         stop=(g == 1), perf_mode=DR)
                    nc.vector.tensor_copy(
                        v8v[:, j, :, 0:DK],
                        ps.rearrange("p (h x) -> p h x", x=DK))
                nc.vector.tensor_copy(
                    v8v[:, :, :, DK:DK + 2],
                    ones8_sb.rearrange("p (j h x) -> p j h x", j=2, x=2))

        # ================= attention =================
        with tc.tile_pool(name="attp", bufs=1) as apool:
            # ctxT8[s]: [128, (j, n)] fp8; head h -> tile h//4, part (h%2)*64,
            # j = (h//2)%2
            ctxT8_sb = [apool.tile([P, 2 * NQ], F8, tag=f"cx{i}",
                                   name=f"ctxT8_{i}") for i in range(2)]
            with tc.tile_pool(name="psatt", bufs=1, space="PSUM") as psa:
                for hp in range(4):
                    ctx_ps = [psa.tile([P, NQ], F32, tag="pc", bufs=2,
                                       name="ctx_ps") for _ in range(2)]
                    at8_q = [[None, None] for _ in range(NSTEP)]

                    def scores_exp_mask(s):
                        for i in range(2):
                            po = i * DK
                            sp = psa.tile([P, 1024], F32, tag="ps", bufs=3,
                                          name="sc_ps")
                            for j in range(2):
                                mt = 2 * s + j
                                nc.tensor.matmul(
                                    sp[:, j * NQ:(j + 1) * NQ],
                                    kT_sb[hp][po:po + DK, mt * P:(mt + 1) * P],
                                    qT_sb[hp][po:po + DK, :],
                                    start=True, stop=True)
                            at = apool.tile([P, 1024], BF, tag="at", bufs=4,
                                            name="at")
                            nc.scalar.activation(at[:], sp[:], AF.Exp,
                                                 scale=0.125)
                            at8 = apool.tile([P, 1024], F8, tag="at8", bufs=4,
                                             name="at8")
                            nc.vector.tensor_mul(
                                at8[:], at[:],
                                mask_sb[:, s * 1024:(s + 1) * 1024])
                            at8_q[s][i] = at8

                    def ctx_step(s):
                        for i, h in ((0, 2 * hp), (1, 2 * hp + 1)):
                            v8v = v8_sb[s].rearrange("p (j h x) -> p j h x",
                                                     j=2, x=DK + 2)
                            at8v = at8_q[s][i].rearrange("p (j n) -> p j n",
                                                         j=2)
                            nc.tensor.matmul(
                                ctx_ps[i][0:DK + 2, :], v8v[:, :, h, :],
                                at8v[:], start=(s == 0), stop=(s == NSTEP - 1),
                                perf_mode=DR)

                    # software pipeline: ctx of step s issues after the
                    # scores/exp/mask of step s+1, so the PE queue never
                    # parks on a ctx that waits for ACT/DVE
                    scores_exp_mask(0)
                    for s in range(1, NSTEP):
                        scores_exp_mask(s)
                        ctx_step(s - 1)
                    ctx_step(NSTEP - 1)

                    # normalize: ctxT8_h = ctx[0:64] * approx(1/rowsum)
                    for i in range(2):
                        h = 2 * hp + i
                        st, j = h // 4, (h // 2) % 2
                        rsum = apool.tile([1, NQ], F32, tag="rsum", bufs=2,
                                          name="rsum")
                        nc.vector.tensor_copy(rsum[:], ctx_ps[i][DK:DK + 1, :])
                        rec = apool.tile([1, NQ], F32, tag="rec", bufs=2,
                                         name="rec")
                        nc.vector.reciprocal_approx_fast(rec[:], rsum[:])
                        bc = apool.tile([P, NQ], F32, tag="bc", bufs=2,
                                        name="bc")
                        nc.gpsimd.partition_broadcast(bc[0:DK, :], rec[:])
                        dstv = ctxT8_sb[st].rearrange("p (j n) -> p j n", j=2)
                        if h % 2 == 0:
                            nc.vector.tensor_mul(dstv[0:DK, j, :],
                                                 ctx_ps[i][0:DK, :],
                                                 bc[0:DK, :])
                        else:
                            tmp8 = apool.tile([P, NQ], F8, tag="tmp8", bufs=2,
                                              name="tmp8")
                            nc.vector.tensor_mul(tmp8[0:DK, :],
                                                 ctx_ps[i][0:DK, :],
                                                 bc[0:DK, :])
                            nc.sync.dma_start(dstv[DK:P, j, :], tmp8[0:DK, :])

            # ---- Wo + residual + LN1 + transpose ----
            h1_sb = [h1pool.tile([P, D], F32, tag=f"h1_{i}", name=f"h1_{i}")
                     for i in range(4)]
            # h1T8[g]: [128, (j, q)] fp8, channel c = g*256 + j*128 + p
            h1T8_sb = [h1pool.tile([P, 2 * NQ], F8, tag=f"h1T{i}",
                                   name=f"h1T8_{i}") for i in range(2)]
            wo8v = wo8_sb.rearrange("p (s j e) -> p s j e", s=2, j=2)
            with tc.tile_pool(name="pspost", bufs=2, space="PSUM") as psw:
                for qt in range(4):
                    ps = psw.tile([P, D], F32, tag="po", name="wo_ps")
                    for s in range(2):
                        ctxv = ctxT8_sb[s].rearrange("p (j n) -> p j n", j=2)
                        nc.tensor.matmul(ps[:],
                                         ctxv[:, :, qt * P:(qt + 1) * P],
                                         wo8v[:, s], start=(s == 0),
                                         stop=(s == 1), perf_mode=DR)
                    h1 = _layer_norm(nc, h1pool, qt, ps, hq_sb[qt],
                                     ln_sb["g1"], ln_sb["b1l"], h1_sb[qt],
                                     eps_sb)
                    h1b = h1pool.tile([P, D], BF, tag="h1b", bufs=2,
                                      name="h1b")
                    nc.vector.tensor_copy(h1b[:], h1[:])
                    for ct in range(4):
                        tp = psw.tile([P, P], BF, tag="tp", name="tp")
                        nc.tensor.transpose(tp[:], h1b[:, ct * P:(ct + 1) * P],
                                            identb_sb[:])
                        h1T8v = h1T8_sb[ct // 2].rearrange(
                            "p (j q) -> p j q", j=2)
                        nc.vector.tensor_copy(
                            h1T8v[:, ct % 2, qt * P:(qt + 1) * P], tp[:])

        qkv_es.close()

        # ================= FFN =================
        with tc.tile_pool(name="ffnp", bufs=1) as fpool, \
             tc.tile_pool(name="psffn", bufs=4, space="PSUM") as psf:
            w1_sb = fpool.tile([P, 4 * DFF], F8, tag="w18")
            nc.sync.dma_start(w1_sb[:], w18[:])
            w2_sb = fpool.tile([P, 16 * D], F8, tag="w28")
            nc.sync.dma_start(w2_sb[:], w28[:])
            w1v = w1_sb.rearrange("p (g j f) -> p g j f", g=2, j=2)
            w2v = w2_sb.rearrange("p (fp j e) -> p fp j e", fp=8, j=2)
            # fT8[fp]: [128, (j, q)] fp8, f-channel = fp*256 + j*128 + p
            fT8_sb = [fpool.tile([P, 2 * NQ], F8, tag=f"fT{i}",
                                 name=f"fT8_{i}") for i in range(8)]
            for ft in range(16):
                ps = psf.tile([P, NQ], F32, tag="pf", name="f_ps")
                for g in range(2):
                    h1T8v = h1T8_sb[g].rearrange("p (j q) -> p j q", j=2)
                    nc.tensor.matmul(ps[:],
                                     w1v[:, g, :, ft * P:(ft + 1) * P],
                                     h1T8v[:], start=(g == 0), stop=(g == 1),
                                     perf_mode=DR)
                fv = fT8_sb[ft // 2].rearrange("p (j q) -> p j q", j=2)
                nc.vector.tensor_scalar(fv[:, ft % 2, :], ps[:],
                                        b1_sb[:, ft:ft + 1], 0.0,
                                        op0=ALU.add, op1=ALU.max)
            for qt in range(4):
                ps = psf.tile([P, D], F32, tag="pf", name="ff_ps")
                for fp in range(8):
                    fv = fT8_sb[fp].rearrange("p (j q) -> p j q", j=2)
                    nc.tensor.matmul(ps[:], fv[:, :, qt * P:(qt + 1) * P],
                                     w2v[:, fp], start=(fp == 0), stop=False,
                                     perf_mode=DR)
                nc.tensor.matmul(ps[:], ones_sb[:], b2_sb[:],
                                 start=False, stop=True)
                h2 = _layer_norm(nc, h1pool, qt + 4, ps, h1_sb[qt],
                                 ln_sb["g2"], ln_sb["b2l"], None, eps_sb)
                nc.sync.dma_start(out[qt * P:(qt + 1) * P, :], h2[:])


def _layer_norm(nc, pool, uid, z_ps, res_sb, g_sb, b_sb, out_tile, eps_sb):
    """out = LN(z_ps + res_sb) * g + b, f32. Returns the output tile."""
    x = pool.tile([P, D], F32, tag=f"x{uid % 4}", bufs=2, name=f"x{uid}")
    s1 = pool.tile([P, 1], F32, tag="s1", bufs=4, name="s1")
    nc.vector.scalar_tensor_tensor(x[:], z_ps[:], 0.0, res_sb[:],
                                   op0=ALU.add, op1=ALU.add, accum_out=s1[:])
    xsq = pool.tile([P, D], F32, tag="xsq", bufs=2, name="xsq")
    s2 = pool.tile([P, 1], F32, tag="s2", bufs=4, name="s2")
    nc.vector.tensor_mul(xsq[:], x[:], x[:])
    nc.vector.reduce_sum(s2[:], xsq[:], axis=mybir.AxisListType.X)
    nm = pool.tile([P, 1], F32, tag="nm", bufs=4, name="nm")
    nc.vector.tensor_scalar_mul(nm[:], s1[:], -1.0 / D)
    m2 = pool.tile([P, 1], F32, tag="m2", bufs=4, name="m2")
    nc.vector.tensor_mul(m2[:], nm[:], nm[:])
    var = pool.tile([P, 1], F32, tag="var", bufs=4, name="var")
    nc.vector.scalar_tensor_tensor(var[:], s2[:], 1.0 / D, m2[:],
                                   op0=ALU.mult, op1=ALU.subtract)
    # rstd = rsqrt(var + eps), pure DVE: bit-trick seed + 3 Newton steps
    I32 = mybir.dt.int32
    ve = pool.tile([P, 1], F32, tag="ve", bufs=4, name="ve")
    nc.vector.tensor_scalar_add(ve[:], var[:], eps_sb[:])
    rstd = pool.tile([P, 1], F32, tag="rstd", bufs=4, name="rstd")
    nc.vector.tensor_single_scalar(rstd[:].bitcast(I32), ve[:].bitcast(I32),
                                   1, op=ALU.arith_shift_right)
    nc.vector.tensor_single_scalar(rstd[:].bitcast(I32), rstd[:].bitcast(I32),
                                   0x5F3759DF, op=ALU.subtract)
    nc.vector.tensor_single_scalar(rstd[:].bitcast(I32), rstd[:].bitcast(I32),
                                   -1, op=ALU.mult)
    tq = pool.tile([P, 1], F32, tag="tq", bufs=4, name="tq")
    for _ in range(3):
        nc.vector.tensor_mul(tq[:], rstd[:], rstd[:])
        nc.vector.tensor_mul(tq[:], tq[:], ve[:])
        nc.vector.tensor_scalar_mul(tq[:], tq[:], -0.5)
        nc.vector.tensor_scalar_add(tq[:], tq[:], 1.5)
        nc.vector.tensor_mul(rstd[:], rstd[:], tq[:])
    # xn = (x - mean) * rstd, in place
    nc.vector.tensor_scalar_add(x[:], x[:], nm[:])
    nc.vector.tensor_scalar_mul(x[:], x[:], rstd[:])
    if out_tile is None:
        out_tile = pool.tile([P, D], F32, tag=f"x{uid % 4}", bufs=2,
                             name=f"h2_{uid}")
    nc.vector.tensor_mul(out_tile[:], x[:], g_sb[:])
    nc.vector.tensor_add(out_tile[:], out_tile[:], b_sb[:])
    return out_tile


def _fold_dr(x, ngroups):
    """[C, cols] -> [128, ngroups*2*cols]: out[p, (g, j, c)] = x[g*256+j*128+p, c]."""
    C, cols = x.shape
    assert C == ngroups * 256
    y = x.reshape(ngroups, 2, P, cols).transpose(2, 0, 1, 3)
    return np.ascontiguousarray(y.reshape(P, ngroups * 2 * cols))


def _prep_inputs(inputs):
    h = np.asarray(inputs["h"], np.float32)
    adj = np.asarray(inputs["adj"])
    f32 = np.float32

    def bf(x):
        return np.ascontiguousarray(np.asarray(x, np.float32).astype(BF16))

    def f8(x):
        return np.ascontiguousarray(np.asarray(x, np.float32).astype(F8NP))

    hT = np.ascontiguousarray(h.T)  # [D, N]
    adjb = (adj != 0)
    np.fill_diagonal(adjb, True)
    adjb_bf = adjb.astype(BF16)

    wq, wk, wv, wo = (np.asarray(inputs[k], f32)
                      for k in ("Wq", "Wk", "Wv", "Wo"))
    w1, w2 = np.asarray(inputs["W1"], f32), np.asarray(inputs["W2"], f32)

    # wo8 fold: channel c = (4s + 2j + p//64)*64 + p%64 at [p, (s, j, :)]
    woT = wo.T  # [c, e]
    wo8 = np.empty((P, 2, 2, D), f32)
    for s in range(2):
        for j in range(2):
            for blk in range(2):  # p//64
                c0 = (4 * s + 2 * j + blk) * 64
                wo8[blk * 64:(blk + 1) * 64, s, j, :] = woT[c0:c0 + 64, :]
    wo8 = np.ascontiguousarray(wo8.reshape(P, 4 * D))

    shared = {
        "h8": _fold_dr(f8(hT).astype(F8NP), 2),
        "wq8": _fold_dr(wq.T, 2).astype(F8NP),
        "wk8": _fold_dr(wk.T, 2).astype(F8NP),
        "wv8": _fold_dr(wv.T, 2).astype(F8NP),
        "wo8": wo8.astype(F8NP),
        "w18": _fold_dr(w1.T, 2).astype(F8NP),
        "w28": _fold_dr(w2.T, 8).astype(F8NP),
        "bq": np.ascontiguousarray(np.asarray(inputs["bq"], f32)),
        "bk": np.ascontiguousarray(np.asarray(inputs["bk"], f32)),
        "b1": np.ascontiguousarray(np.asarray(inputs["b1"], f32)),
        "b22": bf(np.asarray(inputs["b2"], f32)[None, :]),
        "g1b": np.ascontiguousarray(
            np.broadcast_to(np.asarray(inputs["ln1_g"], f32), (P, D))),
        "b1b": np.ascontiguousarray(
            np.broadcast_to(np.asarray(inputs["ln1_b"], f32), (P, D))),
        "g2b": np.ascontiguousarray(
            np.broadcast_to(np.asarray(inputs["ln2_g"], f32), (P, D))),
        "b2b": np.ascontiguousarray(
            np.broadcast_to(np.asarray(inputs["ln2_b"], f32), (P, D))),
        "identb": np.eye(P, dtype=f32).astype(BF16),
        "ones8v": np.ones((P, 32), np.float32).astype(F8NP),
    }
    bo = np.asarray(inputs["bo"], f32)
    bv = np.asarray(inputs["bv"], f32)
    # ctx is accumulated without the v bias; fold Wo@bv + bo into the residual
    res_bias = bv @ wo.T + bo
    in_maps = []
    for i in range(NCORES):
        r0 = i * NQ
        m = dict(shared)
        m["hq8"] = _fold_dr(hT[:, r0:r0 + NQ], 2).astype(F8NP)
        m["hq"] = np.ascontiguousarray(h[r0:r0 + NQ, :] + res_bias)
        m["maskT"] = np.ascontiguousarray(adjb_bf[r0:r0 + NQ, :].T)
        in_maps.append(m)
    return in_maps


def kernel(**inputs) -> np.ndarray:
    global _CACHED, LAST_EXEC_NS
    if _CACHED is None:
        _CACHED = _build()
    nc = _CACHED
    in_maps = _prep_inputs(inputs)
    kw = {}
    if TRACE:
        kw = dict(trace=True, tmpdir=TRACE_DIR)
    res = run_bass_kernel_spmd(nc, in_maps, list(range(NCORES)), **kw)
    LAST_EXEC_NS = res.exec_time_ns
    return np.concatenate([res.results[i]["out"] for i in range(NCORES)],
                          axis=0)
